# revision 34
# baseline (speedup 1.0000x reference)
"""DeepSeekMoE forward on 8 TRN2 NeuronCores.

Strategy (expert-parallel, per the sharding hint):
  - Host computes the (tiny) gate: scores = sqrt(softplus(x @ gate_w)),
    top-2 selection, normalized combine weights, and builds per-expert
    token lists (the "all-to-all dispatch" done host-side since kernel()
    receives full inputs and returns the full output).
  - Core e holds routed expert e's weights and processes the tokens
    routed to it (padded to a common capacity C).
  - The shared expert is split along its intermediate dim I across the
    8 cores (each core computes a 384-wide slice for ALL tokens); the
    partial outputs sum to the exact shared-expert output.
  - Host scatters/sums the per-core outputs back to [B, T, D].

Device compute is bf16 (f32 PSUM accumulation): TRN2 PE does bf16 at
1 cycle/row vs 4 for fp32, and bf16 halves the HBM traffic.

Measured on 8 axon TRN2 cores: ~166 us NEFF exec, rel err 4.1e-3.
Perf notes (from perfetto/NTFF iteration):
  - weights stream as host-pre-tiled [128, 6, 128] stationary slabs so
    each DMA is one contiguous run per partition;
  - each sync.dma_start costs ~650 ns of serialized descriptor
    generation (DIRECT2D) on the sync engine, so the startup sequence
    is ordered xg-chunk0, slab0, slab1, xg-chunk1 to get the first
    GEMM1 matmul issued ~9.3 us instead of ~14 us; later-phase
    residents (xt, w2, w2s) are side-loaded mid-loop in a few big
    pieces (engine-order = issue-order does the staggering);
  - 3 dummy matmuls bridge PE-program-ready (~8.0 us) to xg-landed
    (~9.3 us) and start the HAM clock ramp (1.2 -> 2.4 GHz over
    ~3.8 us of sustained PE activity) early;
  - evict-heavy shared GEMM2 units interleave with evict-light routed
    GEMM2 units so the DVE/ACT/DMA eviction pipeline drains under PE;
  - outputs evict as bf16 (host upcasts): rounding the partials costs
    nothing measurable (4.100e-3 vs 4.103e-3 host-sim) and halves the
    output DMA that forms the kernel tail;
  - token chunks are split EQUALLY (272/272 not 512/32): a sliver
    chunk's matmuls are LDWEIGHTS-bound (95 ns load vs 13 ns matmul).
"""

import math

import numpy as np
import ml_dtypes

import concourse.bass as bass
import concourse.tile as tile
from concourse import bacc, mybir
from concourse.bass_utils import run_bass_kernel_spmd

BF16 = np.dtype(ml_dtypes.bfloat16)
DT_BF16 = mybir.dt.bfloat16
DT_F32 = mybir.dt.float32

D = 768            # n_embd
I = 3072           # moe_intermediate_size
E = 8              # n_routed_experts
TOPK = 2
LIMIT = 10.0
NTOK = 2048        # B*T
NCORES = 8
ISH = I // NCORES  # shared-expert I slice per core (384)
DTILES = D // 128  # 6
MI = I // 128      # 24 routed i-tiles
MS = ISH // 128    # 3 shared i-tiles

_BUILD_CACHE: dict = {}
last_results = None  # BassKernelResults of the most recent run (for test.py)


def _chunks(total, step=512):
    # Balanced chunking: a trailing sliver (e.g. 32 wide) makes its
    # matmuls LDWEIGHTS-bound; equal chunks keep every matmul long
    # enough (>= ~128 rows) to hide the stationary loads.
    import math as _m
    n = max(1, _m.ceil(total / step))
    base = total // n
    rem = total - base * n
    out, t0 = [], 0
    for i in range(n):
        ln = base + (1 if i < rem else 0)
        out.append((t0, ln))
        t0 += ln
    return out


def _build(C):
    """Build the SPMD Bass graph for capacity C (tokens per routed expert)."""
    nc = bacc.Bacc("TRN2", target_bir_lowering=False, debug=False)

    TCR = _chunks(C)      # routed token chunks (equal sizes, C % nch == 0)
    TCS = _chunks(NTOK)   # shared token chunks
    DC = _chunks(D)       # output d chunks (512, 256)
    NCH = len(TCR)
    CL = TCR[0][1]
    assert all(tl == CL for _, tl in TCR), (C, TCR)

    ap = lambda name, shape, dt, kind: nc.dram_tensor(name, shape, dt, kind=kind).ap()
    w13 = ap("w13", [2 * MI, 128, DTILES, 128], DT_BF16, "ExternalInput")
    w2 = ap("w2", [128, MI, D], DT_BF16, "ExternalInput")
    w13s = ap("w13s", [2 * MS, 128, DTILES, 128], DT_BF16, "ExternalInput")
    w2s = ap("w2s", [128, MS, D], DT_BF16, "ExternalInput")
    xt = ap("xt", [128, DTILES, NTOK], DT_BF16, "ExternalInput")
    # xg is chunk-major so each chunk's DMA is ONE contiguous run per
    # partition (the startup critical path)
    # layout matches the SBUF tile exactly (per-partition contiguous), so
    # the startup-critical DMA is a single straight copy
    xg = ap("xg", [128, NCH, DTILES, CL], DT_BF16, "ExternalInput")
    out_r = ap("out_r", [D, C], DT_BF16, "ExternalOutput")
    out_s = ap("out_s", [NTOK, D], DT_BF16, "ExternalOutput")

    MIN = mybir.AluOpType.min
    MAX = mybir.AluOpType.max
    SILU = mybir.ActivationFunctionType.Silu
    COPY = mybir.ActivationFunctionType.Copy

    with tile.TileContext(nc) as tc:
        with (
            tc.tile_pool(name="res", bufs=1) as res,
            tc.tile_pool(name="slab", bufs=8) as slabs,
            tc.tile_pool(name="tmp", bufs=4) as tmps,
            tc.tile_pool(name="ev", bufs=4) as evs,
            tc.tile_pool(name="ps", bufs=8, space="PSUM") as ps1,
        ):
            ps2 = ps1
            # xg chunk 0 first: it gates the very first matmul. Each
            # dma_start costs ~650 ns of serialized sync-engine
            # descriptor generation, so issue order is the priority
            # order: xg-chunk0, slab0, slab1 (via gemm1), xg-chunk1
            # (side-load after slab pair 0's DMAs). The other resident
            # tensors are needed only by later phases — their DMAs are
            # issued mid-way through the GEMM1 loop (side_loads) so the
            # startup stream gets the full HBM bandwidth.
            xg_sb = res.tile([128, NCH, DTILES, CL], DT_BF16)

            # PE warm-up: bridge PE-program-ready (~8.0 us) to
            # xg-chunk0-landed (~9.6 us) with dummy matmuls, ZERO GAP:
            # the HAM clock governor latches its pstate based on early
            # activity — a 2.3 us idle right after short warm-ups capped
            # the whole kernel at 1.96 GHz (197 us vs 166 us measured).
            warm = res.tile([128, 512], DT_BF16)
            nc.vector.memset(warm[:], 0.0)
            pw = ps1.tile([128, 512], DT_F32, tag="ps", name="pw")

            def warmup():
                # 6 x 512 rows ~ 2.8 us at the 1.2 GHz cold clock: bridges
                # to the first real matmul's data (~1.2 MB of gated DMA
                # lands ~10.8 us) so real work starts near full clock.
                for i in range(6):
                    nc.tensor.matmul(pw[:], warm[:, :128], warm[:],
                                     start=(i == 0), stop=(i == 5))
            xt_sb = res.tile([128, DTILES, NTOK], DT_BF16)
            w2_sb = res.tile([128, MI, D], DT_BF16)
            w2s_sb = res.tile([128, MS, D], DT_BF16)
            h_sb = res.tile([128, MI, C], DT_BF16)
            hs_sb = res.tile([128, MS, NTOK], DT_BF16)

            def gemm1(npairs, wsrc, xa, tchunks, hout, side_loads={},
                      slab_eng=None):
                # hout[i, t] = silu(min(W1.T x, L)) * clip(W3.T x, -L, L)
                # xa(ci, d, t0, tl) -> [128, tl] moving AP for chunk ci.
                # side_loads[m] fire before slab pair m's DMAs.
                # slab_eng[m] picks the DMA-issuing engine for pair m's
                # slabs: the Activation engine is also a HWDGE, so pair 0's
                # slab descriptors can be generated in parallel with the
                # sync engine's xg descriptors (~650 ns each, serialized
                # per engine).
                for m in range(npairs):
                    for fn in side_loads.get(m, []):
                        fn()
                    eng = (slab_eng or {}).get(m, nc.sync)
                    sg = slabs.tile([128, DTILES, 128], DT_BF16, tag="slab")
                    eng.dma_start(sg[:], wsrc[2 * m])
                    su = slabs.tile([128, DTILES, 128], DT_BF16, tag="slab")
                    eng.dma_start(su[:], wsrc[2 * m + 1])
                    for ci, (t0, tl) in enumerate(tchunks):
                        pg = ps1.tile([128, 512], DT_F32, tag="ps", name="pg")[:, :tl]
                        for d in range(DTILES):
                            nc.tensor.matmul(
                                pg[:], sg[:, d, :], xa(ci, d, t0, tl),
                                start=(d == 0), stop=(d == DTILES - 1))
                        pu = ps1.tile([128, 512], DT_F32, tag="ps", name="pu")[:, :tl]
                        for d in range(DTILES):
                            nc.tensor.matmul(
                                pu[:], su[:, d, :], xa(ci, d, t0, tl),
                                start=(d == 0), stop=(d == DTILES - 1))
                        # bf16 intermediates: ~2x DVE/ACT throughput, and h
                        # is bf16 anyway so the extra rounding is free
                        tg = tmps.tile([128, 512], DT_BF16, tag="tg", name="tg")[:, :tl]
                        nc.vector.tensor_scalar(tg[:], pg[:], LIMIT, None, MIN)
                        sa = tmps.tile([128, 512], DT_BF16, tag="sa", name="sa")[:, :tl]
                        nc.scalar.activation(sa[:], tg[:], SILU)
                        tu = tmps.tile([128, 512], DT_BF16, tag="tu", name="tu")[:, :tl]
                        nc.vector.tensor_scalar(tu[:], pu[:], LIMIT, -LIMIT, MIN, MAX)
                        nc.vector.tensor_mul(hout[:, m, t0:t0 + tl], sa[:], tu[:])

            def gemm2T_units(nitiles, h, w2sb, tlen_total, dst):
                # dst[d, t] = w2.T @ h — transposed output layout; PE cost
                # scales with tlen_total itself, not its 128-padded tiles.
                # The combine-weight scaling happens on the host instead.
                for (t0, tl) in _chunks(tlen_total):
                    for dt_ in range(DTILES):
                        def unit(t0=t0, tl=tl, dt_=dt_):
                            ps = ps2.tile([128, 512], DT_F32, tag="ps", name="pt")[:, :tl]
                            for m in range(nitiles):
                                nc.tensor.matmul(
                                    ps[:], w2sb[:, m, dt_ * 128:(dt_ + 1) * 128],
                                    h[:, m, t0:t0 + tl],
                                    start=(m == 0), stop=(m == nitiles - 1))
                            ev = evs.tile([128, 512], DT_BF16, tag="ev", name="ev")[:, :tl]
                            # alternate both the copy engine and the DMA-
                            # issuing engine (SP and ACT are both HWDGEs) so
                            # neither descriptor-generation path serializes
                            if dt_ % 2 == 0:
                                nc.vector.tensor_copy(ev[:], ps[:])
                                nc.scalar.dma_start(
                                    dst[dt_ * 128:(dt_ + 1) * 128, t0:t0 + tl], ev[:])
                            else:
                                nc.scalar.activation(ev[:], ps[:], COPY)
                                nc.sync.dma_start(
                                    dst[dt_ * 128:(dt_ + 1) * 128, t0:t0 + tl], ev[:])
                        yield unit

            def gemm2_units(nitiles, h, w2sb, tlen_total, dst):
                # dst[t, d] = h.T @ w2
                for tt, (t0, tl) in enumerate(_chunks(tlen_total, 128)):
                    for di, (d0, dl) in enumerate(DC):
                        def unit(t0=t0, tl=tl, di=di, d0=d0, dl=dl):
                            ps = ps2.tile([128, 512], DT_F32, tag="ps", name="po")[:tl, :dl]
                            for m in range(nitiles):
                                nc.tensor.matmul(
                                    ps[:], h[:, m, t0:t0 + tl],
                                    w2sb[:, m, d0:d0 + dl],
                                    start=(m == 0), stop=(m == nitiles - 1))
                            ev = evs.tile([128, 512], DT_BF16, tag="ev", name="ev")[:tl, :dl]
                            if di % 2 == 0:
                                nc.vector.tensor_copy(ev[:], ps[:])
                                nc.scalar.dma_start(dst[t0:t0 + tl, d0:d0 + dl], ev[:])
                            else:
                                nc.scalar.activation(ev[:], ps[:], COPY)
                                nc.sync.dma_start(dst[t0:t0 + tl, d0:d0 + dl], ev[:])
                        yield unit

            # Interleave the later-phase resident loads into the slab DMA
            # FIFO so they never starve the slab stream. Few big pieces:
            # each dma_start costs ~650 ns of sync-engine time, and a
            # 1.5-4.7 MB transfer only occupies each of the 16 queues for
            # a fraction of one slab-pair's PE time.
            # Small side-load pieces: under the wait-coalescing rule every
            # matmul after a dma_start waits for it, so each piece must
            # land within one slab pair's PE time (~0.5 MB max).
            side = {}
            for j, d in enumerate(range(DTILES)):
                side.setdefault(2 + 2 * j, []).append(
                    lambda d=d: nc.sync.dma_start(xt_sb[:, d, :], xt[:, d, :]))
            for j in range(8):
                side.setdefault(14 + j, []).append(
                    lambda j=j: nc.sync.dma_start(
                        w2_sb[:, 3 * j:3 * (j + 1), :], w2[:, 3 * j:3 * (j + 1), :]))
            side.setdefault(23, []).append(
                lambda: nc.sync.dma_start(w2s_sb[:], w2s[:]))

            # one fused xg DMA (contiguous per partition): all its bytes
            # start flowing at the earliest possible moment, and the first
            # matmul's gate set is {xg, sg0, su0} ~ 1.2 MB
            nc.sync.dma_start(xg_sb[:], xg[:])
            warmup()
            gemm1(MI, w13,
                  lambda ci, d, t0, tl: xg_sb[:, ci, d, :tl],
                  TCR, h_sb, side, slab_eng={0: nc.scalar})
            gemm1(MS, w13s,
                  lambda ci, d, t0, tl: xt_sb[:, d, t0:t0 + tl],
                  TCS, hs_sb)
            # Interleave the evict-heavy shared GEMM2 (many small psum
            # groups) with the evict-light routed GEMM2 (long psum
            # accumulations) so the eviction pipeline drains while PE is
            # still busy, and the kernel ends on an evict-light unit.
            r_units = list(gemm2T_units(MI, h_sb, w2_sb, C, out_r))
            # tiny tail chunks (t-remainder) last: their evictions drain fast
            r_units.sort(key=lambda u: u.__defaults__[0])
            s_units = list(gemm2_units(MS, hs_sb, w2s_sb, NTOK, out_s))
            # hold back the final (smallest-eviction, 256-wide) shared unit
            # so the kernel's last eviction chain is the cheapest one
            tail_unit = s_units.pop()
            ns, nr = len(s_units), len(r_units)
            si = 0
            for ri, ru in enumerate(r_units):
                take = (ns * (ri + 1)) // nr
                while si < min(take, ns):
                    s_units[si]()
                    si += 1
                ru()
            while si < ns:
                s_units[si]()
                si += 1
            tail_unit()

    nc.compile()
    return nc


def _slabify(w):
    """[768, ncols] -> [ncols//128, 128, 6, 128] stationary slabs.

    slab[m, p, a, f] = w[a*128 + p, m*128 + f]
    """
    ncols = w.shape[1]
    return np.ascontiguousarray(
        w.reshape(DTILES, 128, ncols // 128, 128).transpose(2, 1, 0, 3))


def _ptile(a):
    """[R, cols] with R = n*128 -> [128, n, cols] (partition-major)."""
    r, c = a.shape
    return np.ascontiguousarray(a.reshape(r // 128, 128, c).transpose(1, 0, 2))


def kernel(**inputs) -> np.ndarray:
    global last_results
    x = np.asarray(inputs["x"], dtype=np.float32)
    gate_w = np.asarray(inputs["gate_w"], dtype=np.float32)
    gate_bias = np.asarray(inputs["gate_bias"], dtype=np.float32)
    w1 = np.asarray(inputs["w1"], dtype=np.float32)
    w2 = np.asarray(inputs["w2"], dtype=np.float32)
    w3 = np.asarray(inputs["w3"], dtype=np.float32)
    w1s = np.asarray(inputs["w1s"], dtype=np.float32)
    w2s = np.asarray(inputs["w2s"], dtype=np.float32)
    w3s = np.asarray(inputs["w3s"], dtype=np.float32)

    B, T, _ = x.shape
    N = B * T
    assert N == NTOK, f"kernel compiled for {NTOK} tokens, got {N}"
    flat = x.reshape(N, D)

    # ---- gate (host, f32, mirrors reference semantics) ----
    logits = flat @ gate_w                              # [N, E]
    scores = np.sqrt(np.logaddexp(np.float32(0.0), logits)).astype(np.float32)
    routed = scores + gate_bias
    idx = np.argsort(-routed, axis=1, kind="stable")[:, :TOPK]      # [N, K]
    wts = np.take_along_axis(scores, idx, axis=1)
    wts = wts / np.clip(wts.sum(axis=1, keepdims=True), 1e-6, None)

    # ---- dispatch: per-expert token lists ----
    ee = idx.reshape(-1)
    tok = np.repeat(np.arange(N), TOPK)
    ww = wts.reshape(-1).astype(np.float32)
    toks, cwts, counts = [], [], []
    for e in range(E):
        sel = ee == e
        toks.append(tok[sel])
        cwts.append(ww[sel])
        counts.append(int(sel.sum()))
    # equal-size token chunks of <=512 (PSUM bank limit): pad C so it
    # divides evenly into nch chunks of a multiple of 16
    mx = max(max(counts), 128)
    nch = (mx + 511) // 512
    step = 16 * nch
    C = ((mx + step - 1) // step) * step

    # ---- per-core input maps ----
    xt_h = _ptile(flat.T.astype(BF16))                  # [128, 6, N]
    in_maps = []
    for e in range(E):
        ce = counts[e]
        xg_full = np.zeros((C, D), dtype=np.float32)
        xg_full[:ce] = flat[toks[e]]

        w13 = np.empty((2 * MI, 128, DTILES, 128), dtype=BF16)
        w13[0::2] = _slabify(w1[e].astype(BF16))
        w13[1::2] = _slabify(w3[e].astype(BF16))
        sl = slice(e * ISH, (e + 1) * ISH)
        w13s = np.empty((2 * MS, 128, DTILES, 128), dtype=BF16)
        w13s[0::2] = _slabify(w1s[:, sl].astype(BF16))
        w13s[1::2] = _slabify(w3s[:, sl].astype(BF16))

        in_maps.append({
            "w13": w13,
            "w2": _ptile(w2[e].astype(BF16)),           # [128, 24, 768]
            "w13s": w13s,
            "w2s": _ptile(w2s[sl].astype(BF16)),        # [128, 3, 768]
            "xt": xt_h,
            # [128, NCH, 6, CL]: same iteration order as the SBUF tile
            "xg": np.ascontiguousarray(
                _ptile(xg_full.T.astype(BF16))
                .reshape(128, DTILES, nch, C // nch)
                .transpose(0, 2, 1, 3)),
        })

    # ---- build + run ----
    if C not in _BUILD_CACHE:
        _BUILD_CACHE[C] = _build(C)
    nc = _BUILD_CACHE[C]
    last_results = run_bass_kernel_spmd(nc, in_maps, core_ids=list(range(NCORES)))
    res = last_results.results

    # ---- combine (host): sum shared partials, scatter routed outputs ----
    # device outputs are bf16 partials; upcast before accumulating
    out = res[0]["out_s"].astype(np.float32)
    for c in range(1, NCORES):
        out += res[c]["out_s"].astype(np.float32)
    for e in range(E):
        ce = counts[e]
        if ce:
            out[toks[e]] += (res[e]["out_r"][:, :ce].T.astype(np.float32)
                             * cwts[e][:, None])
    return out.reshape(B, T, D).astype(np.float32)

